# revision 42
# baseline (speedup 1.0000x reference)
"""Trainium2 Bass kernel for nn_CASAtt_MultiHead_v1 (CAS attention block).

Reference computation (per sample):
    qkv = 1x1 conv (qkv_w) -> q, k, v                        [512, 56, 56] each
    q <- SE(dwconv3x3(q, sq_w, sq_b))   (per-head squeeze-excite)
    k <- SE(dwconv3x3(k, sk_w, sk_b))
    out = proj(dwconv3x3(q + k, dwc_w, dwc_b) * v) + proj_b + x

Distribution: data-parallel over batch, 2 samples per NeuronCore x 8 cores.

v3 design (measured on HW via micro-benchmarks):
* qkv + proj GEMMs in fp8(e4m3) with MatmulPerfMode.DoubleRow
  (553ns per K=512,N=448 output block vs ~750ns bf16).  Full-chain fp8
  emulation on host: rel err ~6e-3 << 2e-2 gate.
* Depthwise convs as diag-matrix matmuls on the PE, packed as 64x64
  tile_position blocks: the two diagonal 64-blocks of a chunk pair
  (oc even/odd) map to 4 distinct array positions by giving the odd
  chunk a half-swap rotation -> 4 concurrent moving streams, measured
  607 Ge/s vs 256 Ge/s for plain 128x128 diag matmuls.  The odd chunks'
  m image ends up half-swapped; all consumers (conv2 weights, v/proj
  GEMM blocks, biases) are host-permuted to match, nothing on-chip
  un-rotates.
* conv2 runs per-chunk on a configurable engine: 'pe' (pair-rotated,
  output back to natural layout), 'dve' (tensor_scalar@4x +
  tensor_tensor@2x chain, ~143 Ge/s), or 'dva' (ACT scale-copies +
  GpSimd tensor_tensor accumulate) to balance engine load.
* SE pooling approximated: mean(dwconv(q)) ~= (sum_taps w)*mean(q)
  (border terms shift s by ~1e-5 of 0.5); mean(q) comes free from the
  accum_out of the q/k GEMM drains, so conv1 outputs never materialize:
  m = dw3_q(q)*s_q + dw3_k(k)*s_k accumulates all 18 taps of both
  branches into one PSUM group with s folded into the diag weights.
* o2 = (conv2+b)*v is built by a single STT/TT per tile directly into a
  [128, 4, NPIX] fp8 tile (DoubleRow moving operand for proj).
* Residual + output in bf16 (abs budget 0.109 at absmax 5.45; bf16
  costs ~0.011); host converts the bf16 output back to f32.
* Mixed-dtype tensor-tensor DVE ops (psum f32 + bf16) NaN on HW; all
  tensor-tensor ops keep operand dtypes equal.
"""

import numpy as np

DIM = 512
NH = 4
HD = 128
HD4 = 32
B, H, W = 16, 56, 56
N_CORES = 8
BL = B // N_CORES

TH = 8                  # rows per tile
NT = H // TH            # 7
TN = TH * W             # 448
WP = W + 2              # 58 padded row stride
TPAD = TH * WP          # 464
PADN = (H + 2) * WP     # 3364
NPIX = H * W            # 3136

TAPS = [(dy, dx) for dy in (-1, 0, 1) for dx in (-1, 0, 1)]
NTAP = 9


def default_cfg():
    return dict(
        qkv_fp8=1,
        proj_fp8=1,
        # per-sample, per-chunk conv2 engine; 'pe' chunks must come in
        # (even, odd) pairs.  Sample 0's dve chains overlap sample 1's
        # mconv on the PE; the last sample stays all-pe for a clean tail.
        conv2_assign0='pe,pe,dve,dve',
        conv2_assign1='pe,pe,pe,pe',
        mconv_G=2,
        conv2_G=1,
    )


# layout helpers ------------------------------------------------------------

def _lay_m(oc):
    """channel-within-chunk at partition p of m[oc] (PE 64-block rot)."""
    p = np.arange(HD)
    return 64 * ((p // 64 - oc) % 2) + p % 64


def _lay_id(oc):
    return np.arange(HD)


def layouts(cfg):
    c2a_b = [cfg['conv2_assign0'].split(','), cfg['conv2_assign1'].split(',')]
    lay_m = [_lay_m(oc) for oc in range(NH)]          # m buffer layout
    lay_c2_b = []                                     # per-sample o2/v layout
    for b in range(BL):
        lay = []
        for oc in range(NH):
            if c2a_b[b % 2][oc] == 'pe':
                lay.append(_lay_id(oc))               # pair rotation undoes
            else:
                lay.append(lay_m[oc])                 # per-partition engines
        lay_c2_b.append(lay)
    return c2a_b, lay_m, lay_c2_b


def build_nc(cfg):
    import concourse.bass as bass
    import concourse.mybir as mybir
    import concourse.tile as tile
    from concourse import bacc
    from contextlib import ExitStack

    f32 = mybir.dt.float32
    bf16 = mybir.dt.bfloat16
    fp8 = mybir.dt.float8e4
    AF = mybir.ActivationFunctionType
    AL = mybir.AluOpType
    DR = mybir.MatmulPerfMode.DoubleRow

    xdt = fp8 if cfg['qkv_fp8'] else bf16
    odt = fp8 if cfg['proj_fp8'] else bf16
    c2a_b, _, _ = layouts(cfg)
    MG, CG = cfg['mconv_G'], cfg['conv2_G']

    nc = bacc.Bacc("TRN2", target_bir_lowering=False, debug=False,
                   enable_asserts=False, num_devices=N_CORES)

    # ---------------- DRAM I/O ----------------
    # x/out tiled partition-major: [BL, HD(part), NT, NH, TH, W] so each
    # per-partition DMA run is NH*TH*W contiguous bytes
    x8_d = nc.dram_tensor("x8", [BL, HD, NT, NH, TH, W], xdt,
                          kind="ExternalInput").ap()
    xb_d = nc.dram_tensor("xb", [BL, HD, NT, NH, TH, W], bf16,
                          kind="ExternalInput").ap()
    out_d = nc.dram_tensor("out", [BL, HD, NT, NH, TH, W], bf16,
                           kind="ExternalOutput").ap()
    # consolidated weights, already in SBUF (partition-major) layout
    wqkv_d = nc.dram_tensor("wqkv", [HD, 2, NH, NH, HD], xdt,
                            kind="ExternalInput").ap()
    wv_d = nc.dram_tensor("wvg", [HD, 2, NH, NH, HD], xdt,
                          kind="ExternalInput").ap()
    wp_d = nc.dram_tensor("wp", [HD, 2, NH, NH, HD], odt,
                          kind="ExternalInput").ap()
    dgqk_d = nc.dram_tensor("dgqk", [HD, 2, NH, NTAP, 64], fp8,
                            kind="ExternalInput").ap()
    dg2_d = nc.dram_tensor("dg2", [HD, NH, NTAP, 64], bf16,
                           kind="ExternalInput").ap()
    wv2_d = nc.dram_tensor("wvec2", [HD, NH, NTAP], f32,
                           kind="ExternalInput").ap()
    sew1_d = nc.dram_tensor("sew1", [HD, 2, NH, NH, HD4], f32,
                            kind="ExternalInput").ap()
    sew2_d = nc.dram_tensor("sew2", [HD4, 2, NH, 2, HD], f32,
                            kind="ExternalInput").ap()
    seb1_d = nc.dram_tensor("seb1", [HD4, 2, NH], f32,
                            kind="ExternalInput").ap()
    seb2_d = nc.dram_tensor("seb2", [HD, 2, NH, 2], f32,
                            kind="ExternalInput").ap()
    bias4_d = nc.dram_tensor("bias4", [HD, 5, NH], f32,
                             kind="ExternalInput").ap()

    with tile.TileContext(nc) as tc, ExitStack() as ctx:
        const = ctx.enter_context(tc.tile_pool(name="const", bufs=1))
        big = ctx.enter_context(tc.tile_pool(name="big", bufs=1))
        st = ctx.enter_context(tc.tile_pool(name="st", bufs=2))
        pp = ctx.enter_context(tc.tile_pool(name="pp", bufs=8, space="PSUM"))

        qpad = [big.tile([HD, PADN + 2], fp8, name=f"qpad{c}") for c in range(NH)]
        kpad = [big.tile([HD, PADN + 2], fp8, name=f"kpad{c}") for c in range(NH)]
        mpad = [big.tile([HD, PADN + 2], bf16, name=f"mpad{c}") for c in range(NH)]
        o2all = big.tile([HD, NH, NPIX], odt, name="o2all")
        if any(e != 'pe' for ca in c2a_b for e in ca):
            acc = [big.tile([HD, H * WP], bf16, name=f"acc{i}")
                   for i in range(3)]

        def pad3(t):
            return t[:, 1:1 + PADN].rearrange("p (h w) -> p h w", w=WP)

        # zero pad cells: top row(+slop), bottom row(+slop), col pads
        for t in qpad + kpad + mpad:
            nc.vector.memset(t[:, 0:1 + WP + 1], 0.0)
            nc.vector.memset(t[:, 1 + (H + 1) * WP:PADN + 2], 0.0)
            nc.vector.memset(
                t[:, 1 + WP + W + 1:1 + WP + W + 1 + H * WP].rearrange(
                    "p (h w) -> p h w", w=WP)[:, :, 0:2], 0.0)

        # ---------- weights (one DMA per group, spread over queues) ----------
        wqkv_sb = const.tile([HD, 2, NH, NH, HD], xdt, name="wqkv_sb")
        nc.sync.dma_start(wqkv_sb, wqkv_d)
        wv_sb = const.tile([HD, 2, NH, NH, HD], xdt, name="wv_sb")
        nc.gpsimd.dma_start(wv_sb, wv_d)
        wp_sb = const.tile([HD, 2, NH, NH, HD], odt, name="wp_sb")
        nc.gpsimd.dma_start(wp_sb, wp_d)
        gemm_w = {}
        for bi, nm in enumerate(('q', 'k')):
            for oc in range(NH):
                gemm_w[nm, oc] = wqkv_sb[:, bi, oc]
        for b in range(BL):
            for oc in range(NH):
                gemm_w['v', b, oc] = wv_sb[:, b % 2, oc]
                gemm_w['p', b, oc] = wp_sb[:, b % 2, oc]
        dgqk_sb = const.tile([HD, 2, NH, NTAP, 64], fp8, name="dgqk_sb")
        nc.scalar.dma_start(dgqk_sb, dgqk_d)
        dg2_sb = const.tile([HD, NH, NTAP, 64], bf16, name="dg2_sb")
        nc.scalar.dma_start(dg2_sb, dg2_d)
        dg = {}
        for bi, key in enumerate(('q', 'k')):
            for oc in range(NH):
                dg[key, oc] = dgqk_sb[:, bi, oc]
        for oc in range(NH):
            dg['2', oc] = dg2_sb[:, oc]
        dgs = {}
        for key in ('q', 'k'):
            for oc in range(NH):
                dgs[key, oc] = const.tile([HD, NTAP, 64], fp8,
                                          name=f"dgs{key}{oc}")
        wvec2_sb = const.tile([HD, NH, NTAP], f32, name="wvec2_sb")
        nc.gpsimd.dma_start(wvec2_sb, wv2_d)
        wvec2 = {oc: wvec2_sb[:, oc] for oc in range(NH)}

        sew1_sb = const.tile([HD, 2, NH, NH, HD4], f32, name="sew1_sb")
        nc.scalar.dma_start(sew1_sb, sew1_d)
        sew2_sb = const.tile([HD4, 2, NH, 2, HD], f32, name="sew2_sb")
        nc.scalar.dma_start(sew2_sb, sew2_d)
        seb1_sb = const.tile([HD4, 2, NH], f32, name="seb1_sb")
        nc.gpsimd.dma_start(seb1_sb, seb1_d)
        seb2_sb = const.tile([HD, 2, NH, 2], f32, name="seb2_sb")
        nc.gpsimd.dma_start(seb2_sb, seb2_d)
        bias4_sb = const.tile([HD, 5, NH], f32, name="bias4_sb")
        nc.scalar.dma_start(bias4_sb, bias4_d)
        se = {}
        for br in range(2):
            for oc in range(NH):
                se['w1', br, oc] = sew1_sb[:, br, oc]  # [128, NH, HD4]
                se['b1', br, oc] = seb1_sb[:, br, oc:oc + 1]
                se['w2n', br, oc] = sew2_sb[:, br, oc, 0]
                se['w2r', br, oc] = sew2_sb[:, br, oc, 1]
                se['b2n', br, oc] = seb2_sb[:, br, oc, 0:1]
                se['b2r', br, oc] = seb2_sb[:, br, oc, 1:2]
        bqr = [bias4_sb[:, 0, oc:oc + 1] for oc in range(NH)]
        bkr = [bias4_sb[:, 1, oc:oc + 1] for oc in range(NH)]
        dwcb = [bias4_sb[:, 2, oc:oc + 1] for oc in range(NH)]
        projb = [bias4_sb[:, 3, oc:oc + 1] for oc in range(NH)]
        dwcb_m = [bias4_sb[:, 4, oc:oc + 1] for oc in range(NH)]

        def tap_sl(buf, t, j):
            dy, dx = TAPS[j]
            base = 1 + (t * TH + 1) * WP + dy * WP + dx
            return buf[:, base:base + TPAD]

        def gemm(ps, wtile, xtile, tsl, dr):
            if dr:
                for ks in range(0, NH, 2):
                    nc.tensor.matmul(ps, wtile[:, ks:ks + 2, :],
                                     xtile[:, ks:ks + 2, tsl],
                                     start=(ks == 0), stop=(ks == 2),
                                     perf_mode=DR)
            else:
                for kc in range(NH):
                    nc.tensor.matmul(ps, wtile[:, kc, :], xtile[:, kc, tsl],
                                     start=(kc == 0), stop=(kc == NH - 1))

        # ================= pipelined per-sample phases =================
        # PE FIFO per sample b:
        #   warmup | A(b) | V(b) | SE(b) | conv2(b-1) | proj(b-1) | mconv(b)
        # conv2/proj of the previous sample fill the PE while SE(b)'s
        # pooled->sigmoid->weight-scale chain resolves on ACT/DVE, so the
        # PE never idles long enough for the HAM clock gate to re-throttle.
        zwarm = const.tile([HD, 64], bf16, name="zwarm")
        nc.vector.memset(zwarm, 0.0)

        def warmup(n, tag):
            ps = pp.tile([HD, 16], f32, tag="ps", name=f"warm{tag}")
            for i in range(n):
                nc.tensor.matmul(ps[0:64, :], zwarm[:, 0:64],
                                 zwarm[:, 0:16], start=(i == 0),
                                 stop=(i == n - 1), skip_group_check=True)

        def ph_a(b):
            # xstats: per-tile sums of x for the linearized SE pooling
            # (mean(q) = Wq @ mean(x)); drains split across ACT and DVE
            xstats = st.tile([HD, NH, NT], f32, name=f"xstats{b}",
                             tag="xstats")
            for t in range(NT):
                r0 = t * TH
                xt = st.tile([HD, NH, TN], xdt, tag="xt", bufs=3,
                             name=f"xt{b}_{t}")
                nc.sync.dma_start(
                    xt.rearrange("p c (h w) -> p c h w", w=W),
                    x8_d[b, :, t])
                nc.vector.tensor_reduce(xstats[:, :, t:t + 1], xt,
                                        mybir.AxisListType.X, AL.add)
                for br, (nm, dst) in enumerate((('q', qpad), ('k', kpad))):
                    for oc in range(NH):
                        ps = pp.tile([HD, TN], f32, tag="ps",
                                     name=f"g{b}_{br}_{t}_{oc}")
                        gemm(ps, gemm_w[nm, oc], xt, slice(None),
                             cfg['qkv_fp8'])
                        dsl = pad3(dst[oc])[:, 1 + r0:1 + r0 + TH, 1:1 + W]
                        psl = ps.rearrange("p (h w) -> p h w", w=W)
                        nc.scalar.copy(dsl, psl)
            return xstats

        def ph_v(b):
            vall = st.tile([HD, NH, NPIX], bf16, tag="vall", bufs=2,
                           name=f"vall{b}")
            for t in range(NT):
                r0 = t * TH
                xt = st.tile([HD, NH, TN], xdt, tag="xt", bufs=3,
                             name=f"xtv{b}_{t}")
                nc.sync.dma_start(
                    xt.rearrange("p c (h w) -> p c h w", w=W),
                    x8_d[b, :, t])
                for oc in range(NH):
                    ps = pp.tile([HD, TN], f32, tag="ps", name=f"v{b}_{t}_{oc}")
                    gemm(ps, gemm_w['v', b, oc], xt, slice(None),
                         cfg['qkv_fp8'])
                    nc.scalar.copy(vall[:, oc, t * TN:(t + 1) * TN], ps)
            return vall

        def ph_se_front(b, stats):
            # pooled from x sums via linearity; first SE layer on PE
            pooled_x = const.tile([HD, NH, 1], f32, tag="pooled", bufs=2,
                                  name=f"poolx{b}")
            nc.vector.tensor_reduce(pooled_x, stats,
                                    mybir.AxisListType.X, AL.add)
            ps1 = {}
            for br in range(2):
                for oc in range(NH):
                    ps1[br, oc] = pp.tile([HD4, 1], f32, tag="ps",
                                          name=f"se1_{b}_{br}_{oc}")
                    for kc in range(NH):
                        nc.tensor.matmul(ps1[br, oc],
                                         se['w1', br, oc][:, kc, :],
                                         pooled_x[:, kc, :],
                                         start=(kc == 0), stop=(kc == NH - 1))
            return ps1

        def ph_se_back(b, ps1):
            hvec = {}
            ps2 = {}
            sv = {}
            for br in range(2):
                for oc in range(NH):
                    h = const.tile([HD4, 1], f32, tag="hvec", bufs=8,
                                   name=f"h{b}_{br}_{oc}")
                    nc.scalar.activation(h, ps1[br, oc], AF.Relu,
                                         bias=se['b1', br, oc])
                    hvec[br, oc] = h
            for br in range(2):
                for oc in range(NH):
                    p2 = pp.tile([HD, 2], f32, tag="ps",
                                 name=f"se2_{b}_{br}_{oc}")
                    nc.tensor.matmul(p2[:, 0:1], se['w2n', br, oc],
                                     hvec[br, oc], start=True, stop=True,
                                     skip_group_check=True)
                    nc.tensor.matmul(p2[:, 1:2], se['w2r', br, oc],
                                     hvec[br, oc], start=True, stop=True,
                                     skip_group_check=True)
                    ps2[br, oc] = p2
            warmup(10, f"seb{b}")
            for br in range(2):
                for oc in range(NH):
                    s_nat = const.tile([HD, 1], f32, tag="s_nat", bufs=8,
                                       name=f"sn{b}_{br}_{oc}")
                    nc.scalar.activation(s_nat, ps2[br, oc][:, 0:1],
                                         AF.Sigmoid, bias=se['b2n', br, oc])
                    sr = const.tile([HD, 1], f32, tag="s_rot", bufs=8,
                                    name=f"sr{b}_{br}_{oc}")
                    nc.scalar.activation(sr, ps2[br, oc][:, 1:2],
                                         AF.Sigmoid, bias=se['b2r', br, oc])
                    sv[br, oc] = (s_nat, sr)
            for br in range(2):
                for oc in range(NH):
                    key = 'q' if br == 0 else 'k'
                    nc.vector.tensor_scalar(dgs[key, oc], dg[key, oc],
                                            sv[br, oc][0], None, AL.mult)
            bias_m = []
            for oc in range(NH):
                tmp = const.tile([HD, 1], f32, tag="bm_tmp", bufs=2,
                                 name=f"bmt{b}_{oc}")
                nc.vector.tensor_scalar(tmp, bqr[oc], sv[0, oc][1],
                                        None, AL.mult)
                bm = const.tile([HD, 1], f32, tag="bias_m", bufs=4,
                                name=f"bm{b}_{oc}")
                nc.vector.scalar_tensor_tensor(bm, bkr[oc], sv[1, oc][1], tmp,
                                               AL.mult, AL.add)
                bias_m.append(bm)
            return bias_m

        def ph_mconv(b, bias_m):
            for g in range(0, NT, MG):
                gts = list(range(g, min(g + MG, NT)))
                pst = [[pp.tile([HD, TPAD], f32, tag="ps",
                                name=f"m{b}_{t}_{oc}") for oc in range(NH)]
                       for t in gts]
                for jj in range(2 * NTAP):
                    br, j = divmod(jj, NTAP)
                    key, srcb = ('q', qpad) if br == 0 else ('k', kpad)
                    for pair in range(2):
                        for cc in range(2):
                            oc = 2 * pair + cc
                            for a in range(2):
                                bb = (a + cc) % 2
                                for ti in range(len(gts)):
                                    nc.tensor.matmul(
                                        pst[ti][oc][64 * bb:64 * bb + 64, :],
                                        dgs[key, oc][64 * a:64 * a + 64, j, :],
                                        tap_sl(srcb[oc], gts[ti], j)[
                                            64 * a:64 * a + 64, :],
                                        start=(jj == 0),
                                        stop=(jj == 2 * NTAP - 1),
                                        tile_position=(64 * a, 64 * bb),
                                        skip_group_check=True)
                for ti, t in enumerate(gts):
                    for oc in range(NH):
                        nc.scalar.activation(
                            pad3(mpad[oc])[:, 1 + t * TH:1 + t * TH + TH,
                                           1:1 + W],
                            pst[ti][oc].rearrange(
                                "p (h w) -> p h w", w=WP)[:, :, 1:1 + W],
                            AF.Identity, bias=bias_m[oc])

        def ph_conv2(b, vall):
            c2a = c2a_b[b % 2]
            pe_ocs = [oc for oc in range(NH) if c2a[oc] == 'pe']
            for g in range(0, NT, CG):
                gts = list(range(g, min(g + CG, NT)))
                if pe_ocs:
                    pst = [{oc: pp.tile([HD, TPAD], f32, tag="ps",
                                        name=f"c2{b}_{t}_{oc}")
                            for oc in pe_ocs} for t in gts]
                    for j in range(NTAP):
                        for oc in pe_ocs:
                            cc = oc % 2
                            for bb in range(2):
                                b2 = (bb + cc) % 2
                                for ti in range(len(gts)):
                                    nc.tensor.matmul(
                                        pst[ti][oc][64 * b2:64 * b2 + 64, :],
                                        dg['2', oc][64 * bb:64 * bb + 64, j, :],
                                        tap_sl(mpad[oc], gts[ti], j)[
                                            64 * bb:64 * bb + 64, :],
                                        start=(j == 0), stop=(j == NTAP - 1),
                                        tile_position=(64 * bb, 64 * b2),
                                        skip_group_check=True)
                    for ti, t in enumerate(gts):
                        for oc in pe_ocs:
                            c2t = st.tile([HD, TN], bf16, tag="c2t", bufs=3,
                                          name=f"c2t{b}_{t}_{oc}")
                            nc.scalar.activation(
                                c2t.rearrange("p (h w) -> p h w", w=W),
                                pst[ti][oc].rearrange(
                                    "p (h w) -> p h w", w=WP)[:, :, 1:1 + W],
                                AF.Identity, bias=dwcb[oc])
                            nc.vector.tensor_tensor(
                                o2all[:, oc, t * TN:(t + 1) * TN], c2t,
                                vall[:, oc, t * TN:(t + 1) * TN], AL.mult)
            for oc in range(NH):
                eng = c2a[oc]
                if eng == 'pe':
                    continue
                wv2 = wvec2[oc]
                cur, nxt, tmp = 0, 1, 2
                dy, dx = TAPS[0]
                base = 1 + WP + dy * WP + dx
                cnt = H * WP
                nc.vector.tensor_scalar(
                    acc[cur][:, 0:cnt],
                    mpad[oc][:, base:base + cnt], wv2[:, 0:1], None, AL.mult)
                for j in range(1, NTAP):
                    dy, dx = TAPS[j]
                    base = 1 + WP + dy * WP + dx
                    msl = mpad[oc][:, base:base + cnt]
                    if eng == 'dve':
                        nc.vector.tensor_scalar(
                            acc[tmp][:, 0:cnt], msl, wv2[:, j:j + 1],
                            None, AL.mult)
                        nc.vector.tensor_tensor(
                            acc[nxt][:, 0:cnt], acc[cur][:, 0:cnt],
                            acc[tmp][:, 0:cnt], AL.add)
                    else:
                        nc.scalar.activation(
                            acc[tmp][:, 0:cnt], msl, AF.Copy,
                            scale=wv2[:, j:j + 1])
                        nc.gpsimd.tensor_tensor(
                            acc[nxt][:, 0:cnt], acc[cur][:, 0:cnt],
                            acc[tmp][:, 0:cnt], AL.add)
                    cur, nxt, tmp = nxt, tmp, cur
                acc3 = acc[cur][:, 0:cnt].rearrange("p (h w) -> p h w", w=WP)
                nc.vector.scalar_tensor_tensor(
                    o2all[:, oc, :].rearrange("p (h w) -> p h w", w=W),
                    acc3[:, :, 1:1 + W], dwcb_m[oc],
                    vall[:, oc, :].rearrange("p (h w) -> p h w", w=W),
                    AL.add, AL.mult)

        def ph_proj(b):
            last = (b == BL - 1)
            for t in range(NT):
                r0 = t * TH
                xb = st.tile([HD, NH, TN], bf16, tag="xb", bufs=2,
                             name=f"xb{b}_{t}")
                nc.gpsimd.dma_start(
                    xb.rearrange("p c (h w) -> p c h w", w=W),
                    xb_d[b, :, t])
                ot = st.tile([HD, NH, TN], bf16, tag="ot", bufs=2,
                             name=f"ot{b}_{t}")
                for oc in range(NH):
                    ps = pp.tile([HD, TN], f32, tag="ps", name=f"p{b}_{t}_{oc}")
                    gemm(ps, gemm_w['p', b, oc],
                         o2all.rearrange("p c n -> p c n"),
                         slice(t * TN, (t + 1) * TN), cfg['proj_fp8'])
                    pt = st.tile([HD, TN], bf16, tag="pt", bufs=3,
                                 name=f"pt{b}_{t}_{oc}")
                    if last:
                        # final sample: DVE is the tail bottleneck; ACT is
                        # idle here, and split the residual add across both
                        nc.scalar.activation(pt, ps, AF.Identity,
                                             bias=projb[oc])
                    else:
                        nc.vector.tensor_scalar(pt, ps, projb[oc],
                                                None, AL.add)
                    nc.vector.tensor_tensor(ot[:, oc, :], pt, xb[:, oc, :],
                                            AL.add)
                eng = nc.sync if (last and t % 2 == 0) else nc.gpsimd
                eng.dma_start(
                    out_d[b, :, t],
                    ot.rearrange("p c (h w) -> p c h w", w=W))

        warmup(150, "t0")
        prev = None
        for b in range(BL):
            stats = ph_a(b)
            ps1 = ph_se_front(b, stats)
            bias_m = ph_se_back(b, ps1)
            vall = ph_v(b)
            if prev is not None:
                ph_conv2(prev[0], prev[1])
            ph_mconv(b, bias_m)
            if prev is not None:
                ph_proj(prev[0])
            prev = (b, vall)
        ph_conv2(prev[0], prev[1])
        ph_proj(prev[0])

    nc.compile()
    return nc


# ---------------------------------------------------------------------------
# host-side weight prep
# ---------------------------------------------------------------------------

def prep_weights(inputs, cfg):
    import ml_dtypes
    f32 = np.float32
    bf = ml_dtypes.bfloat16
    e4 = ml_dtypes.float8_e4m3
    xdt = e4 if cfg['qkv_fp8'] else bf
    odt = e4 if cfg['proj_fp8'] else bf
    c2a_b, lay_m, lay_c2_b = layouts(cfg)

    qkv_w = np.asarray(inputs['qkv_w'], f32)
    proj_w = np.asarray(inputs['proj_w'], f32)

    def gemm_blocks(wmat, row_perm, dt):
        # lhsT blocks [oc][k_part, k_sub, m]; row_perm permutes output chans
        out = np.empty((NH, HD, NH, HD), f32)
        for oc in range(NH):
            rows = wmat[oc * HD:(oc + 1) * HD]
            if row_perm is not None:
                rows = rows[row_perm[oc]]
            out[oc] = rows.reshape(HD, NH, HD).transpose(2, 1, 0)
        return np.ascontiguousarray(out).astype(dt)

    def proj_blocks(wmat, lay_c2):
        # input (o2) channels are in lay_c2 layout: permute columns
        out = np.empty((NH, HD, NH, HD), f32)
        for oc in range(NH):
            cols = wmat[oc * HD:(oc + 1) * HD].reshape(HD, NH, HD)
            perm_cols = np.empty_like(cols)
            for kc in range(NH):
                perm_cols[:, kc, :] = cols[:, kc, lay_c2[kc]]
            out[oc] = perm_cols.transpose(2, 1, 0)
        return np.ascontiguousarray(out)

    def diag_blocks(wconv, row_perm=None):
        w = np.asarray(wconv, f32).reshape(DIM, NTAP)
        out = np.zeros((NH, HD, NTAP, 64), f32)
        for oc in range(NH):
            ch = np.arange(HD) if row_perm is None else row_perm[oc]
            for p in range(HD):
                out[oc, p, :, p % 64] = w[oc * HD + ch[p]]
        return out.astype(bf)

    def wvec_l(wconv):
        # per-partition conv2 weights in the m layout
        w = np.asarray(wconv, f32).reshape(DIM, NTAP)
        out = np.empty((NH, HD, NTAP), f32)
        for oc in range(NH):
            out[oc] = w[oc * HD + lay_m[oc]]
        return out

    npix = float(NPIX)
    wsum_q = np.asarray(inputs['sq_w'], f32).reshape(DIM, NTAP).sum(1)
    wsum_k = np.asarray(inputs['sk_w'], f32).reshape(DIM, NTAP).sum(1)

    def se_prep(w1, b1, w2, b2, wsum, cbias, wbranch):
        # pooled_q = wsum/npix * (Wbranch @ sum_x); fold everything into
        # an effective [512 -> 32] first-layer matrix M1 = (w1*wsum) @ Wb
        w1 = np.asarray(w1, f32)
        b1 = np.asarray(b1, f32)
        w2 = np.asarray(w2, f32)
        b2 = np.asarray(b2, f32)
        cbias = np.asarray(cbias, f32).reshape(NH, HD)
        sew1 = np.empty((NH, DIM, HD4), f32)
        seb1 = np.empty((NH, HD4, 1), f32)
        sew2n = np.empty((NH, HD4, HD), f32)
        seb2n = np.asarray(b2, f32).reshape(NH, HD, 1).copy()
        sew2r = np.empty((NH, HD4, HD), f32)
        seb2r = np.empty((NH, HD, 1), f32)
        for oc in range(NH):
            m1 = (w1[oc] * (wsum[oc * HD:(oc + 1) * HD] / npix)[None, :]) \
                @ wbranch[oc * HD:(oc + 1) * HD]        # [HD4, 512]
            sew1[oc] = m1.T
            seb1[oc] = (b1[oc] + w1[oc] @ cbias[oc]).reshape(HD4, 1)
            sew2n[oc] = w2[oc].T
            r = lay_m[oc]
            sew2r[oc] = w2[oc][r].T
            seb2r[oc] = b2[oc][r].reshape(HD, 1)
        return dict(w1=sew1, b1=seb1, w2n=sew2n, b2n=seb2n, w2r=sew2r,
                    b2r=seb2r)

    sq_b = np.asarray(inputs['sq_b'], f32)
    sk_b = np.asarray(inputs['sk_b'], f32)
    dwc_b = np.asarray(inputs['dwc_b'], f32)
    se_q = se_prep(inputs['cq_w1'], inputs['cq_b1'], inputs['cq_w2'],
                   inputs['cq_b2'], wsum_q, sq_b, qkv_w[0:DIM])
    se_k = se_prep(inputs['ck_w1'], inputs['ck_b1'], inputs['ck_w2'],
                   inputs['ck_b2'], wsum_k, sk_b, qkv_w[DIM:2 * DIM])

    bias4 = np.empty((5, NH, HD), f32)
    for oc in range(NH):
        bias4[0, oc] = sq_b[oc * HD + lay_m[oc]]
        bias4[1, oc] = sk_b[oc * HD + lay_m[oc]]
        # conv2 'pe' chunks add dwc_b at the (natural-layout) drain;
        # dve/dva chunks add it inside the STT in m layout
        bias4[2, oc] = dwc_b[oc * HD:(oc + 1) * HD]
        bias4[4, oc] = dwc_b[oc * HD + lay_m[oc]]
    bias4[3] = np.asarray(inputs['proj_b'], f32).reshape(NH, HD)

    wqkv = np.stack([
        gemm_blocks(qkv_w[0:DIM], None, f32),
        gemm_blocks(qkv_w[DIM:2 * DIM], None, f32)]).astype(xdt)
    wvg = np.stack([
        gemm_blocks(qkv_w[2 * DIM:3 * DIM], lay_c2_b[bb], f32)
        for bb in range(2)]).astype(xdt)
    wpg = np.stack([
        proj_blocks(proj_w, lay_c2_b[bb]) for bb in range(2)]).astype(odt)
    import ml_dtypes as _mld
    dgqk = np.stack([
        diag_blocks(inputs['sq_w']).astype(np.float32),
        diag_blocks(inputs['sk_w']).astype(np.float32)]).astype(
            _mld.float8_e4m3)
    dg2 = diag_blocks(inputs['dwc_w'], row_perm=lay_m)
    sew1 = np.stack([se_q['w1'], se_k['w1']]).reshape(
        2, NH, NH, HD, HD4)                               # [2,NH,kc,HD,HD4]
    sew2 = np.stack([
        np.stack([se_q['w2n'], se_q['w2r']], axis=1),
        np.stack([se_k['w2n'], se_k['w2r']], axis=1)])    # [2,NH,2,HD4,HD]
    seb1 = np.stack([se_q['b1'], se_k['b1']])[..., 0]     # [2,NH,HD4]
    seb2 = np.stack([
        np.stack([se_q['b2n'], se_q['b2r']], axis=1),
        np.stack([se_k['b2n'], se_k['b2r']], axis=1)])[..., 0]  # [2,NH,2,HD]

    # partition-major DRAM layouts (match SBUF tiles; contiguous DMA runs)
    return dict(
        wqkv=np.ascontiguousarray(wqkv.transpose(2, 0, 1, 3, 4)),
        wvg=np.ascontiguousarray(wvg.transpose(2, 0, 1, 3, 4)),
        wp=np.ascontiguousarray(wpg.transpose(2, 0, 1, 3, 4)),
        dgqk=np.ascontiguousarray(dgqk.transpose(2, 0, 1, 3, 4)),
        dg2=np.ascontiguousarray(dg2.transpose(1, 0, 2, 3)),
        wvec2=np.ascontiguousarray(wvec_l(inputs['dwc_w']).transpose(1, 0, 2)),
        sew1=np.ascontiguousarray(sew1.transpose(3, 0, 1, 2, 4)),
        sew2=np.ascontiguousarray(sew2.transpose(3, 0, 1, 2, 4)),
        seb1=np.ascontiguousarray(seb1.transpose(2, 0, 1)),
        seb2=np.ascontiguousarray(seb2.transpose(3, 0, 1, 2)),
        bias4=np.ascontiguousarray(bias4.transpose(2, 0, 1)),
    )


_CACHE = {}


def _get_compiled(cfg_key, cfg):
    if cfg_key not in _CACHE:
        _CACHE[cfg_key] = build_nc(cfg)
    return _CACHE[cfg_key]


def make_in_maps(inputs, cfg):
    import ml_dtypes
    w = prep_weights(inputs, cfg)
    # [B, DIM, H, W] -> [B, HD, NT, NH, TH, W] (partition-major tiling)
    x32 = np.asarray(inputs['x'], np.float32).reshape(
        B, NH, HD, NT, TH, W).transpose(0, 2, 3, 1, 4, 5)
    xdt = ml_dtypes.float8_e4m3 if cfg['qkv_fp8'] else ml_dtypes.bfloat16
    x8 = np.ascontiguousarray(x32).astype(xdt)
    xbf = np.ascontiguousarray(x32).astype(ml_dtypes.bfloat16)
    in_maps = []
    for core in range(N_CORES):
        m = dict(w)
        m['x8'] = x8[core * BL:(core + 1) * BL]
        m['xb'] = xbf[core * BL:(core + 1) * BL]
        in_maps.append(m)
    return in_maps


def gather_out(results):
    out = np.empty((B, DIM, H, W), np.float32)
    for core in range(N_CORES):
        o = np.asarray(results[core]['out'], np.float32)
        # [BL, HD, NT, NH, TH, W] -> [BL, NH*HD, NT*TH, W]
        out[core * BL:(core + 1) * BL] = o.transpose(
            0, 3, 1, 2, 4, 5).reshape(BL, DIM, H, W)
    return out


def kernel(**inputs):
    from concourse import bass_utils
    cfg = default_cfg()
    nc = _get_compiled('main', cfg)
    in_maps = make_in_maps(inputs, cfg)
    res = bass_utils.run_bass_kernel_spmd(nc, in_maps,
                                          core_ids=list(range(N_CORES)))
    return gather_out(res.results)
